# revision 28
# baseline (speedup 1.0000x reference)
"""CosineWeights kernel for Trainium2 (Bass/Tile), SPMD over 8 NeuronCores.

Math (per batch i, head h, memory row j):
    mask2   = mask*mask                                  [H,K]
    proj    = sum_k (mask2*keys)[h,k] * mem[j,k]         [H,J]
    msq     = sum_k mask2[h,k] * mem[j,k]^2              [H,J]
    kn2     = sum_k (mask*keys)^2                        [H]
    sharp   = softplus(str)[h] * proj / sqrt(kn2*msq)    (EPS folded away; norm ~40 >> 1e-6)
    out     = softmax_j(sharp)

Sharding: data-parallel over batch dim (32 batches -> 8 cores x 4), no
cross-core communication.

Schedule (v2): software-pipelined by one mega-tile so the PE never waits
on the copy/square stage, and the epilogue is split per mega-tile so only
a tiny tail remains after the last matmul.

  - prep DMAs (keys/mask/str) are emitted FIRST on sync so they land
    before the memory stream saturates the DMA rings; mega-tile 0 is
    loaded in quarters so the first transposes can start ~2us earlier.
  - memory arrives [j, k] (k contiguous). PE-transposes 128x128 blocks
    into PSUM [k, j]; DVE copies PSUM->SBUF (memT) while ACT squares
    PSUM->SBUF (memT2) -- two independent readers, no serial chain.
  - proj/msq matmuls use [K=128, 32] zero-padded stationary tiles (content
    pre-scaled by softplus(str) resp. kn2) placed at column offset 8*(t%4),
    with tile_position col-group g=t//4, accumulating into one dense
    [128, 512] PSUM tile per batch: partition p = 8*t + h, free = j%512.
  - Per-mega-tile epilogue chunk on rows [32g:32g+32]:
    s = exp(-0.5*ln(msq')) ; sharp = proj'*s ; exp with fused row-sum.
    Per-batch tail (pipelined one extra tile): cross-partition fold/
    broadcast via tiny onehot matmuls, scale, store.
    Softmax has no max-subtraction (|sharp| <= ~6 -> exp is safe in fp32).
"""

import os

import numpy as np

B, H, J, K = 32, 8, 8192, 128
N_CORES = 8
B_LOC = B // N_CORES  # 4

MEGA = 2048            # j elements per mega-tile
NBLK = MEGA // 128     # 128x128 transpose blocks per mega-tile
NT = J // MEGA         # mega-tiles per batch
NQ = MEGA // 512       # 512-wide matmul chunks per mega-tile
T_PER_I = J // 512     # 16 (512-)tiles per batch -> packed 8*t+h on 128 partitions
PQ = 512 // NBLK       # 32

_NC = None
LAST_RESULTS = None
LAST_EXEC_TIME_NS = None


def _kernel_body(ctx, tc, out_d, mem_d, keys_d, str_d, mask_d):
    import concourse.bass as bass
    from concourse import masks, mybir

    nc = tc.nc
    f32 = mybir.dt.float32
    bf16 = mybir.dt.bfloat16
    AF = mybir.ActivationFunctionType

    const_pool = ctx.enter_context(tc.tile_pool(name="const", bufs=1))
    prep_pool = ctx.enter_context(tc.tile_pool(name="prep", bufs=1))
    nat_pool = ctx.enter_context(tc.tile_pool(name="nat", bufs=6))
    memT_pool = ctx.enter_context(tc.tile_pool(name="memT", bufs=3))
    memT2_pool = ctx.enter_context(tc.tile_pool(name="memT2", bufs=3))
    epi_pool = ctx.enter_context(tc.tile_pool(name="epi", bufs=2))
    small_pool = ctx.enter_context(tc.tile_pool(name="small", bufs=2))
    psumT_pool = ctx.enter_context(
        tc.tile_pool(name="psumT", bufs=2, space=bass.MemorySpace.PSUM)
    )
    proj_pool = ctx.enter_context(
        tc.tile_pool(name="projps", bufs=2, space=bass.MemorySpace.PSUM)
    )
    msq_pool = ctx.enter_context(
        tc.tile_pool(name="msqps", bufs=1, space=bass.MemorySpace.PSUM)
    )
    tiny_pool = ctx.enter_context(
        tc.tile_pool(name="tinyps", bufs=1, space=bass.MemorySpace.PSUM)
    )

    M_TILES = B_LOC * NT
    nats = {}

    # ---- prep DMAs first: tiny transfers, must not queue behind the 1MB
    # memory tiles in the DMA rings (they gate the whole prep chain).
    # sync emission: sync's DGE is free early while gpsimd still has the
    # identity build queued ahead of its first big tile emission, so these
    # tiny transfers land before the memory stream saturates the rings.
    IH = B_LOC * H  # 32
    str_sb = prep_pool.tile([IH, 1], f32)
    nc.sync.dma_start(str_sb[:], str_d.rearrange("i h one -> (i h) one"))
    mask_sb = prep_pool.tile([IH, K], f32)
    nc.sync.dma_start(mask_sb[:], mask_d.rearrange("i h k -> (i h) k"))
    keys_sb = prep_pool.tile([IH, K], f32)
    nc.sync.dma_start(keys_sb[:], keys_d.rearrange("i h k -> (i h) k"))

    def issue_load(m, engine, parts=1):
        # partition p holds NBLK consecutive j-rows -> one contiguous
        # DRAM run per partition (peak DMA efficiency). `parts` splits the
        # emission so the first blocks land (and unblock the PE) sooner.
        i, tp = divmod(m, NT)
        nat = nat_pool.tile([128, MEGA], bf16, tag="nat", name=f"nat{m}")
        src = mem_d[i, tp * MEGA : (tp + 1) * MEGA, :].rearrange(
            "(p c) k -> p c k", p=128
        )
        cpp = NBLK // parts  # chunks per part
        for s in range(parts):
            engine.dma_start(
                nat[:, s * cpp * 128 : (s + 1) * cpp * 128].rearrange(
                    "p (c k) -> p c k", c=cpp
                ),
                src[:, s * cpp : (s + 1) * cpp, :],
            )
        nats[m] = nat

    # casting (f32->bf16) DMAs can only be issued by gpsimd. Emit the first
    # two tiles whole (each extra dma_start costs ~630ns of serial DGE
    # emission, so splitting is counterproductive), build the identity while
    # their data streams in, then emit the rest of the prefetch window.
    issue_load(0, nc.gpsimd, parts=2)
    issue_load(1, nc.gpsimd)

    identity_bf = const_pool.tile([128, 128], bf16)
    masks.make_identity(nc, identity_bf[:])
    id8 = const_pool.tile([8, 8], f32)
    masks.make_identity(nc, id8[:])

    issue_load(2, nc.gpsimd)
    issue_load(3, nc.gpsimd)
    issue_load(4, nc.gpsimd)

    # ---- prep: per-(i,h) scalars and stationary matrices --------------------
    mask2 = prep_pool.tile([IH, K], f32)
    nc.vector.tensor_mul(mask2[:], mask_sb[:], mask_sb[:])
    a_t = prep_pool.tile([IH, K], f32)
    nc.vector.tensor_mul(a_t[:], mask2[:], keys_sb[:])
    ak = prep_pool.tile([IH, K], f32)
    nc.vector.tensor_mul(ak[:], a_t[:], keys_sb[:])
    kn2 = prep_pool.tile([IH, 1], f32)
    nc.vector.reduce_sum(kn2[:], ak[:], axis=mybir.AxisListType.X)
    # softplus(x) = ln(1 + e^x); no Softplus ACT table on this build.
    # strengths ~ N(0,1) so e^x is comfortably in fp32 range.
    es = prep_pool.tile([IH, 1], f32)
    nc.scalar.activation(es[:], str_sb[:], AF.Exp)
    sp = prep_pool.tile([IH, 1], f32)
    nc.scalar.activation(sp[:], es[:], AF.Ln, bias=1.0)

    # per-partition scale folded into the cast: one ACT op each
    a_sb = prep_pool.tile([IH, K], bf16)  # softplus(str) * mask^2 * keys
    nc.scalar.activation(a_sb[:], a_t[:], AF.Copy, scale=sp[:])
    b_sb = prep_pool.tile([IH, K], bf16)  # kn2 * mask^2
    nc.scalar.activation(b_sb[:], mask2[:], AF.Copy, scale=kn2[:])

    # transpose [32,128] -> [128,32] on the PE
    prep_ps = tiny_pool.tile([128, 64], bf16, tag="tiny")
    nc.tensor.transpose(prep_ps[:, 0:32], a_sb[:], identity_bf[0:IH, 0:IH])
    nc.tensor.transpose(prep_ps[:, 32:64], b_sb[:], identity_bf[0:IH, 0:IH])

    # zero-padded stationary variants: for (i, o) a [128,32] tile whose
    # columns 8o..8o+8 hold a'_i (resp b'_i); everything else zero.
    lhsA = const_pool.tile([128, B_LOC * 4 * 32], bf16)
    lhsB = const_pool.tile([128, B_LOC * 4 * 32], bf16)
    nc.vector.memset(lhsA[:], 0.0)
    nc.vector.memset(lhsB[:], 0.0)
    # column of (i, o) block = 32*(4i+o) + 8o = 128i + 40o: for fixed o the
    # i-placements are 128-strided, so one 3D-AP copy per o covers all i.
    lhsA_v = lhsA[:].rearrange("p (i r) -> p i r", i=B_LOC)
    lhsB_v = lhsB[:].rearrange("p (i r) -> p i r", i=B_LOC)
    prepA_v = prep_ps[:, 0:32].rearrange("p (i e) -> p i e", i=B_LOC)
    prepB_v = prep_ps[:, 32:64].rearrange("p (i e) -> p i e", i=B_LOC)
    for o in range(4):
        nc.vector.tensor_copy(lhsA_v[:, :, 40 * o : 40 * o + 8], prepA_v)
        nc.scalar.copy(lhsB_v[:, :, 40 * o : 40 * o + 8], prepB_v)

    # fold+broadcast matrix over h = p % 8: Mfold[p, p'] = (p%8 == p'%8),
    # so Mfold^T @ sums gives every partition its head's total in one matmul.
    oneT = const_pool.tile([H, 128], f32)  # oneT[h, 8r+h'] = (h==h')
    for r in range(16):
        nc.vector.tensor_copy(oneT[:, 8 * r : 8 * r + 8], id8[:])
    mf_ps = tiny_pool.tile([128, 128], f32, tag="tiny")
    nc.tensor.matmul(mf_ps[:], oneT[:], oneT[:], start=True, stop=True)
    Mfold = const_pool.tile([128, 128], f32)
    nc.vector.tensor_copy(Mfold[:], mf_ps[:])

    # ---- main loop: software-pipelined stages -------------------------------
    # per-mega-tile state carried between pipeline stages
    st = {}

    def stage_transpose(m):
        nat = nats.pop(m)
        psumT = psumT_pool.tile([128, MEGA], bf16, tag="psumT")
        for b in range(NBLK):
            nc.tensor.transpose(
                psumT[:, b * 128 : (b + 1) * 128],
                nat[:, b * 128 : (b + 1) * 128],
                identity_bf[:],
            )
        # three independent PSUM readers run concurrently: DVE copies, and
        # the square is split between ACT and DVE halves (ACT/DVE cost is
        # driven by free-dim size, so halving the free dim halves the op).
        memT = memT_pool.tile([128, MEGA], bf16, tag="memT")
        nc.vector.tensor_copy(memT[:], psumT[:])
        memT2 = memT2_pool.tile([128, MEGA], bf16, tag="memT2")
        HM = MEGA // 2
        nc.scalar.square(memT2[:, 0:HM], psumT[:, 0:HM])
        nc.vector.tensor_mul(
            memT2[:, HM:MEGA], memT[:, HM:MEGA], memT[:, HM:MEGA]
        )
        st[m] = (memT, memT2)

    def stage_matmul(m):
        i, tp = divmod(m, NT)
        memT, memT2 = st.pop(m)
        if tp == 0:
            st[("proj", i)] = proj_pool.tile([128, 512], f32, tag="proj", name=f"proj{i}")
            st[("msq", i)] = msq_pool.tile([128, 512], f32, tag="msq", name=f"msq{i}")
        proj_ps = st[("proj", i)]
        msq_ps = st[("msq", i)]

        memT_v = memT[:].rearrange("kk (b pp) -> kk b pp", b=NBLK)
        memT2_v = memT2[:].rearrange("kk (b pp) -> kk b pp", b=NBLK)
        # the final tile streams in two column halves so the last batch's
        # epilogue can start while the second half is still on the PE
        col_parts = 1
        BH = NBLK // col_parts
        for cp in range(col_parts):
            bsl = slice(cp * BH, (cp + 1) * BH)
            osl = slice(cp * BH * PQ, (cp + 1) * BH * PQ)
            for q in range(NQ):
                t = tp * NQ + q
                g, o = divmod(t, 4)
                v = i * 4 + o
                # rhs columns for the j-run [512q, 512q+512): psum_T col
                # 128b+pp holds j = NBLK*pp + b, so take pp in
                # [PQ*q, PQ*q+PQ) across all b.
                nc.tensor.matmul(
                    proj_ps[32 * g : 32 * g + 32, osl],
                    lhsA[:, v * 32 : (v + 1) * 32],
                    memT_v[:, bsl, PQ * q : PQ * q + PQ],
                    start=(o == 0),
                    stop=(o == 3),
                    tile_position=(0, 32 * g),
                )
                nc.tensor.matmul(
                    msq_ps[32 * g : 32 * g + 32, osl],
                    lhsB[:, v * 32 : (v + 1) * 32],
                    memT2_v[:, bsl, PQ * q : PQ * q + PQ],
                    start=(o == 0),
                    stop=(o == 3),
                    tile_position=(0, 32 * g),
                )

    def stage_epilogue(i, nsplit=2):
        # per-batch epilogue on the dense [128,512] tiles, pipelined in
        # free-dim pieces so the dependent ACT/DVE chain latency shrinks
        # (ACT/DVE op cost scales with free-dim size; full 128 partitions
        # are kept -- never split below 128 partitions).
        proj_ps = st.pop(("proj", i))
        msq_ps = st.pop(("msq", i))
        PF = 512 // nsplit
        et = epi_pool.tile([128, 512], f32, tag="et", name=f"et{i}")
        sums2 = small_pool.tile([128, 4], f32, tag="sums2", name=f"sums2_{i}")
        if nsplit < 4:
            nc.vector.memset(sums2[:, nsplit:4], 0.0)
        for hf in range(nsplit):
            sl = slice(PF * hf, PF * hf + PF)
            lnm = epi_pool.tile([128, PF], f32, tag="lnm", name=f"lnm{i}_{hf}")
            nc.scalar.activation(lnm[:], msq_ps[:, sl], AF.Ln)
            s_t = epi_pool.tile([128, PF], f32, tag="s_t", name=f"s_t{i}_{hf}")
            nc.scalar.activation(s_t[:], lnm[:], AF.Exp, scale=-0.5)
            sharp = epi_pool.tile(
                [128, PF], f32, tag="sharp", name=f"sharp{i}_{hf}"
            )
            nc.vector.tensor_mul(sharp[:], proj_ps[:, sl], s_t[:])
            nc.scalar.activation(
                et[:, sl], sharp[:], AF.Exp, accum_out=sums2[:, hf : hf + 1]
            )
        sums = small_pool.tile([128, 1], f32, tag="sums", name=f"sums{i}")
        nc.vector.reduce_sum(sums[:], sums2[:], axis=mybir.AxisListType.X)
        st[("et", i)] = et
        st[("sums", i)] = sums

    def stage_tail(i, last=False):
        # per-h totals folded AND broadcast in one matmul: Mfold^T @ sums
        et = st.pop(("et", i))
        sums = st.pop(("sums", i))
        hb_ps = tiny_pool.tile([128, 1], f32, tag="tiny")
        nc.tensor.matmul(hb_ps[:], Mfold[:], sums[:], start=True, stop=True)
        rb = small_pool.tile([128, 1], f32, tag="rb")
        nc.vector.reciprocal(rb[:], hb_ps[:])

        # et free index f = 32*b + pp corresponds to j = 16*pp + b within
        # the row's 512-j run; permute while applying the softmax scale.
        # For the last batch, scale+store in two pieces so the first DMA
        # emission overlaps the second scale.
        out_t = epi_pool.tile([128, 512], f32, tag="out_t")
        parts = 2 if last else 1
        PP = (NBLK * PQ // PQ) // parts  # pp values per part (32/parts)
        for cp in range(parts):
            psl = slice(cp * (PQ // parts), (cp + 1) * (PQ // parts))
            osl = slice(cp * (512 // parts), (cp + 1) * (512 // parts))
            nc.vector.tensor_scalar_mul(
                out_t[:, osl].rearrange("r (pp b) -> r b pp", b=NBLK),
                et[:].rearrange("r (b pp) -> r b pp", b=NBLK)[:, :, psl],
                rb[:],
            )
            nc.sync.dma_start(
                out_d[i].rearrange("h (t f) -> t h f", t=T_PER_I)[:, :, osl],
                out_t[:, osl],
            )

    for m in range(M_TILES):
        if m + 5 < M_TILES:
            issue_load(m + 5, nc.gpsimd)
        stage_transpose(m)
        if m >= 1:
            stage_matmul(m - 1)
        if m >= 1 and (m - 1) % NT == NT - 1:
            stage_epilogue((m - 1) // NT)
        if m >= 2 and (m - 2) % NT == NT - 1:
            stage_tail((m - 2) // NT)
    stage_matmul(M_TILES - 1)
    stage_epilogue(B_LOC - 1)
    stage_tail(B_LOC - 1)


def _patch_act_tables():
    """The ACT table-load inserter maps each activation to the first set
    containing it; by default Exp lands in exp_and_others and Ln in
    natural_log, forcing a ~1.5us table switch per Ln<->Exp transition
    (2 per batch epilogue). Reorder so the combined
    natural_log_exp_and_others set is found first -- table loads resolve
    by name, so reordering is safe."""
    import concourse.bacc as bacc

    if getattr(bacc, "_cosine_act_tables_patched", False):
        return
    orig = bacc.get_activation_tables

    def patched(arch):
        from concourse import mybir as _mb

        tables = dict(orig(arch))
        if "natural_log_exp_and_others" not in tables:
            return tables
        # Keep dict order/indices identical (act_func_set_id indexes the
        # act_info.json order); just stop Exp/Ln resolving to the
        # single-function sets so both land in the combined set.
        drop = {_mb.ActivationFunctionType.Exp, _mb.ActivationFunctionType.Ln}
        for name in list(tables):
            if name == "natural_log_exp_and_others":
                continue
            fns = tables[name]
            if isinstance(fns, (set, frozenset)) and (fns & drop):
                tables[name] = fns - drop
        return tables

    bacc.get_activation_tables = patched
    bacc._cosine_act_tables_patched = True


def _build():
    from contextlib import ExitStack

    import concourse.bacc as bacc
    import concourse.tile as tile
    from concourse import mybir

    _patch_act_tables()

    nc = bacc.Bacc(
        "TRN2",
        target_bir_lowering=False,
        debug=False,
        num_devices=N_CORES,
        num_swdge_queues=2,
    )
    f32 = mybir.dt.float32
    mem_d = nc.dram_tensor("memory", [B_LOC, J, K], f32, kind="ExternalInput").ap()
    keys_d = nc.dram_tensor("keys", [B_LOC, H, K], f32, kind="ExternalInput").ap()
    str_d = nc.dram_tensor(
        "strengths", [B_LOC, H, 1], f32, kind="ExternalInput"
    ).ap()
    mask_d = nc.dram_tensor("mask", [B_LOC, H, K], f32, kind="ExternalInput").ap()
    out_d = nc.dram_tensor("out", [B_LOC, H, J], f32, kind="ExternalOutput").ap()

    with tile.TileContext(nc) as tc:
        with ExitStack() as ctx:
            _kernel_body(ctx, tc, out_d, mem_d, keys_d, str_d, mask_d)

    nc.compile()
    return nc


def get_nc():
    global _NC
    if _NC is None:
        _NC = _build()
    return _NC


def kernel(memory, keys, strengths, mask):
    global LAST_RESULTS, LAST_EXEC_TIME_NS
    from concourse.bass_utils import run_bass_kernel_spmd

    nc = get_nc()
    in_maps = []
    for c in range(N_CORES):
        sl = slice(c * B_LOC, (c + 1) * B_LOC)
        in_maps.append(
            {
                "memory": np.ascontiguousarray(memory[sl], dtype=np.float32),
                "keys": np.ascontiguousarray(keys[sl], dtype=np.float32),
                "strengths": np.ascontiguousarray(strengths[sl], dtype=np.float32),
                "mask": np.ascontiguousarray(mask[sl], dtype=np.float32),
            }
        )
    res = run_bass_kernel_spmd(nc, in_maps, list(range(N_CORES)))
    LAST_RESULTS = res
    LAST_EXEC_TIME_NS = res.exec_time_ns
    out = np.concatenate([res.results[c]["out"] for c in range(N_CORES)], axis=0)
    return out.astype(np.float32, copy=False)
